# revision 3
# baseline (speedup 1.0000x reference)
import math
from functools import partial

import numpy as np

B, L, D = 4, 4096, 512
H, DK = 8, 64
R = 64              # DFT radix (L = R*R)
MA = 25
PAD = (MA - 1) // 2  # 12
EPS = 1e-5
HALF = 2048
SLICE = 2072        # 2048 + 24 halo (clamped at sequence edges)
SOFF = 2024         # slice start for half 1
TOPK = 8

_FN = None


def _consts():
    i = np.arange(R)
    C64 = np.cos(2 * np.pi * np.outer(i, i) / R).astype(np.float32)
    S64 = np.sin(2 * np.pi * np.outer(i, i) / R).astype(np.float32)
    e, j = np.meshgrid(i, i, indexing="ij")
    TwC = np.cos(2 * np.pi * e * j / L).astype(np.float32)   # [e, j]
    TwS = np.sin(2 * np.pi * e * j / L).astype(np.float32)
    return C64, S64, TwC, TwS


def _build():
    global _FN
    if _FN is not None:
        return _FN
    import jax
    import jax.numpy as jnp
    from jax import lax

    C64_, S64_, TwC_, TwS_ = _consts()

    def percore(x_b, Wq, bq, Wk, bk, Wv, bv, Wo, bo, W1, b1, W2, b2, g1, be1, g2, be2):
        C64, S64, TwC, TwS = map(jnp.asarray, (C64_, S64_, TwC_, TwS_))
        cid = lax.axis_index("c")
        half = cid % 2
        hoff = half * (H // 2)

        def headslice(W, b):
            Wr = W.reshape(D, H, DK)
            Ws = lax.dynamic_slice(Wr, (0, hoff, 0), (D, H // 2, DK))
            br = b.reshape(H, DK)
            bs = lax.dynamic_slice(br, (hoff, 0), (H // 2, DK))
            return Ws, bs

        Wqs, bqs = headslice(Wq, bq)
        Wks, bks = headslice(Wk, bk)
        Wvs, bvs = headslice(Wv, bv)

        Q = jnp.einsum("ld,dhk->hlk", x_b, Wqs) + bqs[:, None, :]   # [4,L,dk]
        K = jnp.einsum("ld,dhk->hlk", x_b, Wks) + bks[:, None, :]
        V = jnp.einsum("ld,dhk->hlk", x_b, Wvs) + bvs[:, None, :]

        def fwd(X):  # [4, L, dk] -> Z(re,im) [4, e, g, dk]
            M = X.reshape(H // 2, R, R, DK)                       # t = 64 i + j
            Yre = jnp.einsum("ie,hijd->hejd", C64, M)
            Yim = -jnp.einsum("ie,hijd->hejd", S64, M)
            Tc = TwC[None, :, :, None]
            Ts = TwS[None, :, :, None]
            Ypre = Yre * Tc + Yim * Ts
            Ypim = Yim * Tc - Yre * Ts
            Zre = jnp.einsum("hejd,jg->hegd", Ypre, C64) + jnp.einsum("hejd,jg->hegd", Ypim, S64)
            Zim = jnp.einsum("hejd,jg->hegd", Ypim, C64) - jnp.einsum("hejd,jg->hegd", Ypre, S64)
            return Zre, Zim

        Qre, Qim = fwd(Q)
        Kre, Kim = fwd(K)
        Sre = jnp.einsum("hegd,hegd->heg", Qre, Kre) + jnp.einsum("hegd,hegd->heg", Qim, Kim)
        Sim = jnp.einsum("hegd,hegd->heg", Qim, Kre) - jnp.einsum("hegd,hegd->heg", Qre, Kim)

        Ure = jnp.einsum("heg,ga->hea", Sre, C64) - jnp.einsum("heg,ga->hea", Sim, S64)
        Uim = jnp.einsum("heg,ga->hea", Sim, C64) + jnp.einsum("heg,ga->hea", Sre, S64)
        Tc2 = TwC[None]   # Tw2[e, A] == Tw[e, j] grid (same table)
        Ts2 = TwS[None]
        Upre = Ure * Tc2 - Uim * Ts2
        Upim = Uim * Tc2 + Ure * Ts2
        Cm = jnp.einsum("eb,hea->hba", C64, Upre) - jnp.einsum("eb,hea->hba", S64, Upim)
        corr = Cm.reshape(H // 2, L) * (1.0 / (L * DK))           # tau = 64 b + a

        # manual top-8 (descending) + softmax
        ar = jnp.arange(L, dtype=jnp.int32)
        c = corr
        vals, idxs = [], []
        for _ in range(TOPK):
            m = jnp.max(c, axis=1)
            im = jnp.argmax(c, axis=1).astype(jnp.int32)
            vals.append(m)
            idxs.append(im)
            c = jnp.where(ar[None, :] == im[:, None], -1e30, c)
        tw = jnp.stack(vals, 1)                                   # [4, 8]
        ti = jnp.stack(idxs, 1)
        tw = jax.nn.softmax(tw, axis=-1)

        # delay aggregation: out[h] = sum_k tw[h,k] * roll(V[h], -ti[h,k])
        Vd = jnp.concatenate([V, V], axis=1)                      # [4, 2L, dk]
        outs = []
        for h in range(H // 2):
            acc = jnp.zeros((L, DK), jnp.float32)
            for k in range(TOPK):
                sl = lax.dynamic_slice(Vd[h], (ti[h, k], 0), (L, DK))
                acc = acc + tw[h, k] * sl
            outs.append(acc)
        ctx4 = jnp.stack(outs, 1)                                 # [L, 4, dk]

        ctx = jnp.zeros((L, H, DK), jnp.float32)
        ctx = lax.dynamic_update_slice(ctx, ctx4, (0, hoff, 0))
        ctx = lax.psum(ctx, "c", axis_index_groups=[[0, 1], [2, 3], [4, 5], [6, 7]])
        ctx = ctx.reshape(L, D)

        r0 = half * SOFF
        ctx_s = lax.dynamic_slice(ctx, (r0, 0), (SLICE, D))
        x_s = lax.dynamic_slice(x_b, (r0, 0), (SLICE, D))
        attn = ctx_s @ Wo + bo

        def ln(z, g, b):
            mu = jnp.mean(z, -1, keepdims=True)
            var = jnp.mean((z - mu) ** 2, -1, keepdims=True)
            return (z - mu) * lax.rsqrt(var + EPS) * g + b

        def decomp(z):
            zp = jnp.concatenate(
                [jnp.repeat(z[:1], PAD, 0), z, jnp.repeat(z[-1:], PAD, 0)], 0)
            t = zp[0:SLICE]
            for o in range(1, MA):
                t = t + zp[o:o + SLICE]
            t = t * (1.0 / MA)
            return z - t, t

        x1 = ln(x_s + attn, g1, be1)
        s1, t1 = decomp(x1)
        hmid = jax.nn.gelu(s1 @ W1 + b1, approximate=False)
        ff = hmid @ W2 + b2
        x2 = ln(s1 + ff, g2, be2)
        s2, t2 = decomp(x2)
        tr = t1 + t2

        off = half * (SLICE - HALF)  # 0 or 24
        s2o = lax.dynamic_slice(s2, (off, 0), (HALF, D))
        tro = lax.dynamic_slice(tr, (off, 0), (HALF, D))
        return s2o, tro

    _FN = jax.pmap(percore, axis_name="c", in_axes=(0,) + (None,) * 16)
    return _FN


def kernel(x, Wq, bq, Wk, bk, Wv, bv, Wo, bo, W1, b1, W2, b2, g1, be1, g2, be2):
    fn = _build()
    x = np.asarray(x, np.float32)
    xs = x[np.repeat(np.arange(B), 2)]                            # [8, L, D]
    f32 = lambda w: np.asarray(w, np.float32)
    s2, tr = fn(xs, f32(Wq), f32(bq), f32(Wk), f32(bk), f32(Wv), f32(bv),
                f32(Wo), f32(bo), f32(W1), f32(b1), f32(W2), f32(b2),
                f32(g1), f32(be1), f32(g2), f32(be2))
    s2 = np.asarray(s2).reshape(B, 2 * HALF, D)
    tr = np.asarray(tr).reshape(B, 2 * HALF, D)
    return s2, tr


# revision 4
# speedup vs baseline: 45.0916x; 45.0916x over previous
import math
from functools import partial

import numpy as np

B, L, D = 4, 4096, 512
H, DK = 8, 64
R = 64              # DFT radix (L = R*R)
MA = 25
PAD = (MA - 1) // 2  # 12
EPS = 1e-5
HALF = 2048
SLICE = 2072        # 2048 + 24 halo (clamped at sequence edges)
SOFF = 2024         # slice start for half 1
TOPK = 8

_FN = None
_PERCORE = None


def _consts():
    i = np.arange(R)
    C64 = np.cos(2 * np.pi * np.outer(i, i) / R).astype(np.float32)
    S64 = np.sin(2 * np.pi * np.outer(i, i) / R).astype(np.float32)
    e, j = np.meshgrid(i, i, indexing="ij")
    TwC = np.cos(2 * np.pi * e * j / L).astype(np.float32)   # [e, j]
    TwS = np.sin(2 * np.pi * e * j / L).astype(np.float32)
    return C64, S64, TwC, TwS


def _build():
    global _FN
    if _FN is not None:
        return _FN
    import jax
    import jax.numpy as jnp
    from jax import lax

    C64_, S64_, TwC_, TwS_ = _consts()

    def percore(x_b, Wq, bq, Wk, bk, Wv, bv, Wo, bo, W1, b1, W2, b2, g1, be1, g2, be2):
        C64, S64, TwC, TwS = map(jnp.asarray, (C64_, S64_, TwC_, TwS_))
        cid = lax.axis_index("c")
        half = cid % 2
        hoff = half * (H // 2)

        def headslice(W, b):
            Wr = W.reshape(D, H, DK)
            Ws = lax.dynamic_slice(Wr, (0, hoff, 0), (D, H // 2, DK))
            br = b.reshape(H, DK)
            bs = lax.dynamic_slice(br, (hoff, 0), (H // 2, DK))
            return Ws, bs

        Wqs, bqs = headslice(Wq, bq)
        Wks, bks = headslice(Wk, bk)
        Wvs, bvs = headslice(Wv, bv)

        Q = jnp.einsum("ld,dhk->hlk", x_b, Wqs) + bqs[:, None, :]   # [4,L,dk]
        K = jnp.einsum("ld,dhk->hlk", x_b, Wks) + bks[:, None, :]
        V = jnp.einsum("ld,dhk->hlk", x_b, Wvs) + bvs[:, None, :]

        def fwd(X):  # [4, L, dk] -> Z(re,im) [4, e, g, dk]
            M = X.reshape(H // 2, R, R, DK)                       # t = 64 i + j
            Yre = jnp.einsum("ie,hijd->hejd", C64, M)
            Yim = -jnp.einsum("ie,hijd->hejd", S64, M)
            Tc = TwC[None, :, :, None]
            Ts = TwS[None, :, :, None]
            Ypre = Yre * Tc + Yim * Ts
            Ypim = Yim * Tc - Yre * Ts
            Zre = jnp.einsum("hejd,jg->hegd", Ypre, C64) + jnp.einsum("hejd,jg->hegd", Ypim, S64)
            Zim = jnp.einsum("hejd,jg->hegd", Ypim, C64) - jnp.einsum("hejd,jg->hegd", Ypre, S64)
            return Zre, Zim

        Qre, Qim = fwd(Q)
        Kre, Kim = fwd(K)
        Sre = jnp.einsum("hegd,hegd->heg", Qre, Kre) + jnp.einsum("hegd,hegd->heg", Qim, Kim)
        Sim = jnp.einsum("hegd,hegd->heg", Qim, Kre) - jnp.einsum("hegd,hegd->heg", Qre, Kim)

        Ure = jnp.einsum("heg,ga->hea", Sre, C64) - jnp.einsum("heg,ga->hea", Sim, S64)
        Uim = jnp.einsum("heg,ga->hea", Sim, C64) + jnp.einsum("heg,ga->hea", Sre, S64)
        Tc2 = TwC[None]   # Tw2[e, A] == Tw[e, j] grid (same table)
        Ts2 = TwS[None]
        Upre = Ure * Tc2 - Uim * Ts2
        Upim = Uim * Tc2 + Ure * Ts2
        Cm = jnp.einsum("eb,hea->hba", C64, Upre) - jnp.einsum("eb,hea->hba", S64, Upim)
        corr = Cm.reshape(H // 2, L) * (1.0 / (L * DK))           # tau = 64 b + a

        # manual top-8 (descending) + softmax
        ar = jnp.arange(L, dtype=jnp.int32)
        c = corr
        vals, idxs = [], []
        for _ in range(TOPK):
            m = jnp.max(c, axis=1)
            im = jnp.argmax(c, axis=1).astype(jnp.int32)
            vals.append(m)
            idxs.append(im)
            c = jnp.where(ar[None, :] == im[:, None], -1e30, c)
        tw = jnp.stack(vals, 1)                                   # [4, 8]
        ti = jnp.stack(idxs, 1)
        tw = jax.nn.softmax(tw, axis=-1)

        # delay aggregation: out[h] = sum_k tw[h,k] * roll(V[h], -ti[h,k])
        Vd = jnp.concatenate([V, V], axis=1)                      # [4, 2L, dk]
        outs = []
        for h in range(H // 2):
            acc = jnp.zeros((L, DK), jnp.float32)
            for k in range(TOPK):
                sl = lax.dynamic_slice(Vd[h], (ti[h, k], 0), (L, DK))
                acc = acc + tw[h, k] * sl
            outs.append(acc)
        ctx4 = jnp.stack(outs, 1)                                 # [L, 4, dk]

        ctx = jnp.zeros((L, H, DK), jnp.float32)
        ctx = lax.dynamic_update_slice(ctx, ctx4, (0, hoff, 0))
        ctx = lax.psum(ctx, "c", axis_index_groups=[[0, 1], [2, 3], [4, 5], [6, 7]])
        ctx = ctx.reshape(L, D)

        r0 = half * SOFF
        ctx_s = lax.dynamic_slice(ctx, (r0, 0), (SLICE, D))
        x_s = lax.dynamic_slice(x_b, (r0, 0), (SLICE, D))
        attn = ctx_s @ Wo + bo

        def ln(z, g, b):
            mu = jnp.mean(z, -1, keepdims=True)
            var = jnp.mean((z - mu) ** 2, -1, keepdims=True)
            return (z - mu) * lax.rsqrt(var + EPS) * g + b

        def decomp(z):
            zp = jnp.concatenate(
                [jnp.repeat(z[:1], PAD, 0), z, jnp.repeat(z[-1:], PAD, 0)], 0)
            t = zp[0:SLICE]
            for o in range(1, MA):
                t = t + zp[o:o + SLICE]
            t = t * (1.0 / MA)
            return z - t, t

        x1 = ln(x_s + attn, g1, be1)
        s1, t1 = decomp(x1)
        hmid = jax.nn.gelu(s1 @ W1 + b1, approximate=False)
        ff = hmid @ W2 + b2
        x2 = ln(s1 + ff, g2, be2)
        s2, t2 = decomp(x2)
        tr = t1 + t2

        off = half * (SLICE - HALF)  # 0 or 24
        s2o = lax.dynamic_slice(s2, (off, 0), (HALF, D))
        tro = lax.dynamic_slice(tr, (off, 0), (HALF, D))
        return s2o, tro

    global _PERCORE
    _PERCORE = percore
    _FN = jax.pmap(percore, axis_name="c", in_axes=(0,) + (None,) * 16)
    return _FN


def kernel(x, Wq, bq, Wk, bk, Wv, bv, Wo, bo, W1, b1, W2, b2, g1, be1, g2, be2):
    fn = _build()
    x = np.asarray(x, np.float32)
    xs = x[np.repeat(np.arange(B), 2)]                            # [8, L, D]
    f32 = lambda w: np.asarray(w, np.float32)
    s2, tr = fn(xs, f32(Wq), f32(bq), f32(Wk), f32(bk), f32(Wv), f32(bv),
                f32(Wo), f32(bo), f32(W1), f32(b1), f32(W2), f32(b2),
                f32(g1), f32(be1), f32(g2), f32(be2))
    s2 = np.asarray(s2).reshape(B, 2 * HALF, D)
    tr = np.asarray(tr).reshape(B, 2 * HALF, D)
    return s2, tr
